# revision 1
# baseline (speedup 1.0000x reference)
"""Causal self-attention (B=2, T=2048, C=1024, H=16, rope) on 8 trn2 cores.

Sharding: core i = (batch b = i // 4, head-group g = i % 4 owning heads 4g..4g+3).
Each core computes its 4 heads' attention and a partial projection (transposed);
the host sums the 4 head-group partials per batch and adds b_proj.

On-core dataflow (all matmuls in float32r):
  xT = PE-transpose(x)                                   [C, T]
  qkT = W_qk_perm.T @ x.T   (+bias via K=1 matmul)       [512, T] -> RoPE on evict
  v   = x @ W_v             (+bias)                      [T, 256] natural, + ones col
  S.T = kT.T-slab @ qT-chunk (K=64)                      [128k, 512q] per tile
  expS = exp(S.T / 8)  (ACT), causal-masked on diagonal tiles
  [O.T; l] = sum_k [v | 1].T-slab @ expS                 PV + denominators in one go
  yT = O.T * (1/l)  (1/l broadcast via K=1 ones matmul)
  outT = W_p-shard.T @ yT                                [1024, T] partial, DMA out
"""

import os
import numpy as np

B, T, C, H = 2, 2048, 1024, 16
HS = C // H            # 64
HPC = H // 4           # 4 heads per core
NCORES = 8
TCH = 512              # t/q chunk size
NCH = T // TCH         # 4 chunks
NSLAB = T // 128       # 16 t-slabs

_cache = {}
last_results = None    # BassKernelResults of the most recent run (for test.py)


def _build():
    import concourse.bacc as bacc
    import concourse.mybir as mybir
    import concourse.tile as tile
    from concourse.masks import make_identity

    F32 = mybir.dt.float32
    F32R = mybir.dt.float32r
    AF = mybir.ActivationFunctionType

    nc = bacc.Bacc("TRN2", target_bir_lowering=False, debug=False,
                   num_devices=NCORES)

    x_in = nc.dram_tensor("x_in", (T, C), F32, kind="ExternalInput")
    wqk = nc.dram_tensor("wqk", (C, 512), F32, kind="ExternalInput")
    bqk = nc.dram_tensor("bqk", (1, 512), F32, kind="ExternalInput")
    wv = nc.dram_tensor("wv", (C, 256), F32, kind="ExternalInput")
    bv = nc.dram_tensor("bv", (1, 256), F32, kind="ExternalInput")
    wp = nc.dram_tensor("wp", (256, C), F32, kind="ExternalInput")
    cos_in = nc.dram_tensor("cos_in", (128, T), F32, kind="ExternalInput")
    sin_in = nc.dram_tensor("sin_in", (128, T), F32, kind="ExternalInput")
    cmask = nc.dram_tensor("cmask", (128, 4, TCH), F32, kind="ExternalInput")
    out_t = nc.dram_tensor("out_t", (C, T), F32, kind="ExternalOutput")

    with tile.TileContext(nc) as tc:
        with (
            tc.tile_pool(name="const", bufs=1) as const,
            tc.tile_pool(name="xp", bufs=2) as xp,
            tc.tile_pool(name="xtp", bufs=1) as xtp,
            tc.tile_pool(name="work", bufs=2) as work,
            tc.tile_pool(name="ep", bufs=4) as ep,
            tc.tile_pool(name="ost", bufs=2) as ost,
            tc.tile_pool(name="ps_a", bufs=2, space="PSUM") as ps_a,
            tc.tile_pool(name="ps_s", bufs=3, space="PSUM") as ps_s,
            tc.tile_pool(name="ps_o", bufs=2, space="PSUM") as ps_o,
            tc.tile_pool(name="ps_p", bufs=1, space="PSUM") as ps_p,
        ):
            # ---- constants / weights ----
            ident = const.tile([128, 128], F32)
            make_identity(nc, ident)

            ones_f = const.tile([1, TCH], F32)
            nc.gpsimd.memset(ones_f[:], 1.0)
            ones_r = const.tile([1, TCH], F32R)
            nc.vector.tensor_copy(ones_r[:], ones_f[:])

            wqk_sb = const.tile([128, 8, 512], F32R)
            for hf in range(2):
                stg = work.tile([128, 4, 512], F32, tag="stg")
                nc.sync.dma_start(
                    stg[:], wqk.ap()[hf * 512:(hf + 1) * 512, :]
                    .rearrange("(s p) m -> p s m", p=128))
                nc.vector.tensor_copy(wqk_sb[:, 4 * hf:4 * hf + 4, :], stg[:])
            bqk_sb = const.tile([1, 512], F32R)
            stgb = work.tile([1, 512], F32, tag="stgb")
            nc.sync.dma_start(stgb[:], bqk[:, :])
            nc.vector.tensor_copy(bqk_sb[:], stgb[:])

            wv_sb = const.tile([128, 8, 256], F32R)
            stg = work.tile([128, 8, 256], F32, tag="stg")
            nc.sync.dma_start(stg[:],
                              wv.ap().rearrange("(s p) m -> p s m", p=128))
            nc.vector.tensor_copy(wv_sb[:], stg[:])
            bv_sb = const.tile([1, 256], F32R)
            stgb = work.tile([1, 512], F32, tag="stgb")
            nc.sync.dma_start(stgb[0:1, 0:256], bv[:, :])
            nc.vector.tensor_copy(bv_sb[:], stgb[0:1, 0:256])

            wp_sb = const.tile([128, 2, C], F32R)
            stg = work.tile([128, 2, C], F32, tag="stg")
            nc.sync.dma_start(stg[:],
                              wp.ap().rearrange("(s p) m -> p s m", p=128))
            nc.vector.tensor_copy(wp_sb[:], stg[:])

            cos_sb = const.tile([128, T], F32)
            nc.sync.dma_start(cos_sb[:], cos_in[:, :])
            sin_sb = const.tile([128, T], F32)
            nc.sync.dma_start(sin_sb[:], sin_in[:, :])
            msk_sb = const.tile([128, 4, TCH], F32)
            nc.sync.dma_start(msk_sb[:], cmask[:, :, :])

            # ---- persistent activations ----
            qT = [const.tile([128, T], F32R, name=f"qT{p}", tag=f"qT{p}")
                  for p in range(2)]
            kT = [const.tile([128, T], F32R, name=f"kT{p}", tag=f"kT{p}")
                  for p in range(2)]
            # v with ones column: [t-slab-part, slab, head, 65]
            v_sb = const.tile([128, NSLAB, HPC, 65], F32R)
            ones128 = const.tile([128, 64], F32)
            nc.gpsimd.memset(ones128[:], 1.0)
            nc.vector.tensor_copy(
                v_sb[:, :, :, 64],
                ones128[:, 0:64].rearrange("p (s h) -> p s h", s=NSLAB))

            xT = [xtp.tile([128, TCH], F32R, name=f"xT{c}", tag=f"xT{c}")
                  for c in range(8)]

            for tcH in range(NCH):
                tcols = slice(tcH * TCH, (tcH + 1) * TCH)
                # ======== phase A: x load, transpose, qk(+rope), v ========
                for half in range(2):
                    xin = xp.tile([128, 2, C], F32, tag="xin")
                    r0 = tcH * TCH + half * 256
                    nc.sync.dma_start(
                        xin[:],
                        x_in[r0:r0 + 256, :].rearrange("(s p) m -> p s m", p=128))
                    for ts in range(2):
                        sl = half * 2 + ts  # slab within chunk, 0..3
                        for c in range(8):
                            pt = ps_a.tile([128, 128], F32, tag="a")
                            nc.tensor.transpose(
                                pt[:], xin[:, ts, c * 128:(c + 1) * 128], ident[:])
                            nc.vector.tensor_copy(
                                xT[c][:, sl * 128:(sl + 1) * 128], pt[:])

                # qk matmuls + rope eviction
                for m in range(4):
                    pqk = ps_a.tile([128, TCH], F32, tag="a")
                    for s in range(8):
                        nc.tensor.matmul(pqk[:], wqk_sb[:, s, m * 128:(m + 1) * 128],
                                         xT[s][:], start=(s == 0), stop=False)
                    nc.tensor.matmul(pqk[:], bqk_sb[0:1, m * 128:(m + 1) * 128],
                                     ones_r[0:1, :], start=False, stop=True,
                                     skip_group_check=True)
                    tQ = work.tile([128, TCH], F32, tag="tQ")
                    nc.scalar.activation(tQ[:], pqk[:], AF.Copy)
                    tA = work.tile([128, TCH], F32, tag="tA")
                    tB = work.tile([128, TCH], F32, tag="tB")
                    nc.vector.tensor_mul(tA[:], tQ[:], cos_sb[:, tcols])
                    dest = (qT if m % 2 == 0 else kT)[m // 2]
                    swap = [(0, 32, 32, 64), (32, 64, 0, 32),
                            (64, 96, 96, 128), (96, 128, 64, 96)]
                    for (a0, a1, b0, b1) in swap:
                        nc.vector.tensor_mul(tB[a0:a1, :], tQ[b0:b1, :],
                                             sin_sb[b0:b1, tcols])
                        nc.vector.tensor_add(dest[a0:a1, tcols],
                                             tA[a0:a1, :], tB[a0:a1, :])

                # v matmuls (natural orientation) + eviction into v_sb
                for ts in range(4):
                    pv = ps_a.tile([128, 256], F32, tag="a")
                    for s in range(8):
                        nc.tensor.matmul(pv[:], xT[s][:, ts * 128:(ts + 1) * 128],
                                         wv_sb[:, s, :], start=(s == 0), stop=False)
                    nc.tensor.matmul(pv[:], ones_r[0:1, 0:128], bv_sb[0:1, :],
                                     start=False, stop=True, skip_group_check=True)
                    sl = tcH * 4 + ts
                    nc.scalar.activation(
                        v_sb[:, sl, :, 0:64],
                        pv[:].rearrange("p (h e) -> p h e", e=64), AF.Copy)

                # ======== attention for q-chunk tcH ========
                yT_ch = work.tile([128, 2, TCH], F32R, tag="yT")
                nslabs = 4 * tcH + 4
                for p in range(2):
                    pos = [ps_o.tile([128, TCH], F32, tag="O", name=f"po{hh}")
                           for hh in range(2)]
                    for j in range(nslabs):
                        rr = j - 4 * tcH
                        r = max(rr, 0) * 128  # valid q-suffix start
                        qs = slice(tcH * TCH + r, (tcH + 1) * TCH)
                        for hh in range(2):
                            base, h, po = 64 * hh, 2 * p + hh, pos[hh]
                            psS = ps_s.tile([128, TCH], F32, tag="S")
                            nc.tensor.matmul(
                                psS[:, r:TCH],
                                kT[p][base:base + 64, j * 128:(j + 1) * 128],
                                qT[p][base:base + 64, qs],
                                start=True, stop=True)
                            expS = ep.tile([128, TCH], F32R, tag="expS")
                            nc.scalar.activation(expS[:, r:TCH], psS[:, r:TCH],
                                                 AF.Exp, scale=0.125)
                            if rr >= 0:
                                nc.gpsimd.tensor_mul(expS[:, r:r + 128],
                                        expS[:, r:r + 128],
                                        msk_sb[:, 0, 0:128])
                            nc.tensor.matmul(po[0:65, r:TCH], v_sb[:, j, h, :],
                                             expS[:, r:TCH],
                                             start=(j == 0),
                                             stop=(j == nslabs - 1))
                    for hh in range(2):
                        base, po = 64 * hh, pos[hh]
                        l_f = work.tile([1, TCH], F32, tag="lf")
                        nc.scalar.activation(l_f[:], po[64:65, :], AF.Copy)
                        l_r = work.tile([1, TCH], F32R, tag="lr")
                        with nc.allow_low_precision(reason="f32r rounding of 1/l"):
                            nc.vector.reciprocal(l_r[:], l_f[:])
                        prep = ps_p.tile([64, TCH], F32, tag="P")
                        nc.tensor.matmul(prep[:], ones_r[0:1, 0:64], l_r[0:1, :],
                                         start=True, stop=True)
                        nc.scalar.activation(yT_ch[base:base + 64, p, :],
                                             po[0:64, :], AF.Copy)
                        nc.vector.tensor_mul(yT_ch[base:base + 64, p, :],
                                             yT_ch[base:base + 64, p, :], prep[:])

                # ======== proj for q-chunk tcH ========
                for m in range(8):
                    pp = ps_p.tile([128, TCH], F32, tag="P")
                    for s in range(2):
                        nc.tensor.matmul(pp[:], wp_sb[:, s, m * 128:(m + 1) * 128],
                                         yT_ch[:, s, :], start=(s == 0),
                                         stop=(s == 1))
                    o_st = ost.tile([128, TCH], F32, tag="ost")
                    nc.scalar.activation(o_st[:], pp[:], AF.Copy)
                    nc.sync.dma_start(out_t[m * 128:(m + 1) * 128, tcols], o_st[:])

    nc.compile()
    return nc


def _rope_tables():
    pos = np.arange(T, dtype=np.float32)[:, None]                  # [T, 1]
    i = np.arange(1, HS // 2 + 1, dtype=np.float32)[None]          # [1, 32]
    theta = 1.0 / 10000.0 ** (2.0 * (i - 1.0) / HS)
    ang = pos * theta                                              # [T, 32]
    cos, sin = np.cos(ang).T, np.sin(ang).T                        # [32, T]
    cos_rep = np.tile(cos, (4, 1)).astype(np.float32)              # [128, T]
    sin_sgn = np.concatenate([sin, -sin, sin, -sin], 0).astype(np.float32)
    return cos_rep, sin_sgn


def _masks():
    p = np.arange(128)[:, None]
    f = np.arange(TCH)[None, :]
    m = np.stack([(p <= f - 128 * r) for r in range(4)], axis=1)   # [128, 4, TCH]
    return m.astype(np.float32)


def kernel(x, W_qkv, b_qkv, W_proj, b_proj):
    global last_results
    from concourse.bass_utils import run_bass_kernel_spmd

    if "nc" not in _cache:
        _cache["nc"] = _build()
    nc = _cache["nc"]

    x = np.asarray(x, np.float32)
    W_qkv = np.asarray(W_qkv, np.float32)
    b_qkv = np.asarray(b_qkv, np.float32)
    W_proj = np.asarray(W_proj, np.float32)
    b_proj = np.asarray(b_proj, np.float32)

    perm = np.concatenate([np.arange(0, HS, 2), np.arange(1, HS, 2)])  # even|odd
    cos_rep, sin_sgn = _rope_tables()
    cmask = _masks()

    in_maps = []
    for core in range(NCORES):
        b, g = core // 4, core % 4
        heads = [4 * g + j for j in range(HPC)]
        wq = [W_qkv[:, h * 3 * HS:h * 3 * HS + HS][:, perm] for h in heads]
        wk = [W_qkv[:, h * 3 * HS + HS:h * 3 * HS + 2 * HS][:, perm] for h in heads]
        wv = [W_qkv[:, h * 3 * HS + 2 * HS:h * 3 * HS + 3 * HS] for h in heads]
        bq = [b_qkv[h * 3 * HS:h * 3 * HS + HS][perm] for h in heads]
        bk = [b_qkv[h * 3 * HS + HS:h * 3 * HS + 2 * HS][perm] for h in heads]
        bv_ = [b_qkv[h * 3 * HS + 2 * HS:h * 3 * HS + 3 * HS] for h in heads]
        # col-chunks: [q01 | k01 | q23 | k23]
        wqk = np.concatenate([wq[0], wq[1], wk[0], wk[1],
                              wq[2], wq[3], wk[2], wk[3]], axis=1)
        bqk = np.concatenate([bq[0], bq[1], bk[0], bk[1],
                              bq[2], bq[3], bk[2], bk[3]])[None, :]
        in_maps.append({
            "x_in": np.ascontiguousarray(x[b]),
            "wqk": np.ascontiguousarray(wqk),
            "bqk": np.ascontiguousarray(bqk),
            "wv": np.ascontiguousarray(np.concatenate(wv, axis=1)),
            "bv": np.ascontiguousarray(np.concatenate(bv_)[None, :]),
            "wp": np.ascontiguousarray(W_proj[g * 256:(g + 1) * 256, :]),
            "cos_in": cos_rep,
            "sin_in": sin_sgn,
            "cmask": cmask,
        })

    res = run_bass_kernel_spmd(nc, in_maps, core_ids=list(range(NCORES)))
    last_results = res

    out = np.zeros((B, T, C), dtype=np.float32)
    for core in range(NCORES):
        b = core // 4
        out[b] += res.results[core]["out_t"].T
    out += b_proj[None, None, :]
    return out



# revision 8
# speedup vs baseline: 1.3432x; 1.3432x over previous
"""Causal self-attention (B=2, T=2048, C=1024, H=16, rope) on 8 trn2 cores.

Sharding: core i = (batch b = i // 4, head-group g = i % 4 owning heads 4g..4g+3).
Each core computes its 4 heads' attention and a partial projection (transposed,
bf16); the host sums the 4 head-group partials per batch and adds b_proj.

v2: all matmul operands bf16 (1 cycle/row at any N), x transposed on the host
(no PE transposes / DVE staging), rope as 6 scalar_tensor_tensor ops on DVE
(4x perf mode on all-SBUF bf16), qk psum evict on Pool, exp pairs on Act,
proj evict on Pool, bf16 output.

On-core dataflow:
  xT chunk  = DMA of host-transposed x          [128, 8, 512] bf16
  qkT = W_qk_perm.T @ x.T (+bias K=1 f32r mm)   [512, T] -> rope -> qT/kT bf16
  v   = x @ W_v  (+bias)                        [T, 256], + ones col, bf16
  S.T pair  = kT-slab.T @ qT-chunk (K=64)       [128k, 2, 512q] f32 psum
  expS = exp(S.T / 8) (Act, pair op) -> bf16, causal-masked on diag (Pool)
  [O.T; l] = sum_k [v | 1].T-slab @ expS        PV + denominators, f32 psum
  yT = O.T * (1/l)  (DVE recip + K=1 ones mm broadcast + DVE stt)
  outT = W_p-shard.T @ yT                       [1024, T] bf16 partial, DMA out
"""

import numpy as np

B, T, C, H = 2, 2048, 1024, 16
HS = C // H            # 64
HPC = H // 4           # 4 heads per core
NCORES = 8
TCH = 512              # t/q chunk size
NCH = T // TCH         # 4 chunks
NSLAB = T // 128       # 16 t-slabs

_cache = {}
last_results = None    # BassKernelResults of the most recent run (for test.py)


def _build():
    import concourse.bacc as bacc
    import concourse.mybir as mybir
    import concourse.tile as tile

    F32 = mybir.dt.float32
    F32R = mybir.dt.float32r
    BF16 = mybir.dt.bfloat16
    AF = mybir.ActivationFunctionType
    MUL = mybir.AluOpType.mult
    ADD = mybir.AluOpType.add

    nc = bacc.Bacc("TRN2", target_bir_lowering=False, debug=False,
                   num_devices=NCORES)

    xt_in = nc.dram_tensor("xt_in", (C, T), BF16, kind="ExternalInput")
    wqk = nc.dram_tensor("wqk", (C, 512), BF16, kind="ExternalInput")
    bqk = nc.dram_tensor("bqk", (1, 512), F32R, kind="ExternalInput")
    wv = nc.dram_tensor("wv", (C, 256), BF16, kind="ExternalInput")
    bv = nc.dram_tensor("bv", (1, 256), F32R, kind="ExternalInput")
    wp = nc.dram_tensor("wp", (256, C), BF16, kind="ExternalInput")
    cos_in = nc.dram_tensor("cos_in", (128, T), BF16, kind="ExternalInput")
    sin_in = nc.dram_tensor("sin_in", (128, T), BF16, kind="ExternalInput")
    cmask = nc.dram_tensor("cmask", (128, 128), BF16, kind="ExternalInput")
    out_t = nc.dram_tensor("out_t", (C, T), BF16, kind="ExternalOutput")

    with tile.TileContext(nc) as tc:
        with (
            tc.tile_pool(name="const", bufs=1) as const,
            tc.tile_pool(name="xp", bufs=2) as xp,
            tc.tile_pool(name="work", bufs=3) as work,
            tc.tile_pool(name="ep", bufs=4) as ep,
            tc.tile_pool(name="yp", bufs=2) as yp,
            tc.tile_pool(name="ost", bufs=2) as ost,
            tc.tile_pool(name="ps_a", bufs=2, space="PSUM") as ps_a,
            tc.tile_pool(name="ps_s", bufs=2, space="PSUM") as ps_s,
            tc.tile_pool(name="ps_o", bufs=2, space="PSUM") as ps_o,
        ):
            # ---- constants / weights (direct bf16 DMA, no staging) ----
            wqk_sb = const.tile([128, 8, 512], BF16)
            nc.sync.dma_start(wqk_sb[:],
                              wqk.ap().rearrange("(s p) m -> p s m", p=128))
            wv_sb = const.tile([128, 8, 256], BF16)
            nc.sync.dma_start(wv_sb[:],
                              wv.ap().rearrange("(s p) m -> p s m", p=128))
            wp_sb = const.tile([128, 2, C], BF16)
            nc.scalar.dma_start(wp_sb[:],
                                wp.ap().rearrange("(s p) m -> p s m", p=128))
            bqk_sb = const.tile([1, 512], F32R)
            nc.scalar.dma_start(bqk_sb[:], bqk[:, :])
            bv_sb = const.tile([1, 256], F32R)
            nc.scalar.dma_start(bv_sb[:], bv[:, :])
            cos_sb = const.tile([128, T], BF16)
            nc.gpsimd.dma_start(cos_sb[:], cos_in[:, :])
            sin_sb = const.tile([128, T], BF16)
            nc.gpsimd.dma_start(sin_sb[:], sin_in[:, :])
            msk_sb = const.tile([128, 128], BF16)
            nc.gpsimd.dma_start(msk_sb[:], cmask[:, :])

            ones_f = const.tile([1, TCH], F32)
            nc.gpsimd.memset(ones_f[:], 1.0)
            ones_r = const.tile([1, TCH], F32R)
            nc.vector.tensor_copy(ones_r[:], ones_f[:])

            # ---- persistent activations ----
            qT = [const.tile([128, T], BF16, name=f"qT{p}", tag=f"qT{p}")
                  for p in range(2)]
            kT = [const.tile([128, T], BF16, name=f"kT{p}", tag=f"kT{p}")
                  for p in range(2)]
            # v with ones column: [t-slab-part, slab, head, 65]
            v_sb = const.tile([128, NSLAB, HPC, 65], BF16)
            ones128 = const.tile([128, 64], F32)
            nc.gpsimd.memset(ones128[:], 1.0)
            nc.vector.tensor_copy(
                v_sb[:, :, :, 64],
                ones128[:, 0:64].rearrange("p (s h) -> p s h", s=NSLAB))

            swap = [(0, 32, 32, 64), (32, 64, 0, 32),
                    (64, 96, 96, 128), (96, 128, 64, 96)]

            for tcH in range(NCH):
                tcols = slice(tcH * TCH, (tcH + 1) * TCH)
                # ======== phase A: xT chunk load, qk(+rope), v ========
                xt_ch = xp.tile([128, 8, TCH], BF16, tag="xt")
                nc.sync.dma_start(
                    xt_ch[:],
                    xt_in.ap()[:, tcols].rearrange("(s p) m -> p s m", p=128))

                # qk matmuls + rope eviction
                for m in range(4):
                    pqk = ps_a.tile([128, TCH], F32, tag="a")
                    for s in range(8):
                        nc.tensor.matmul(pqk[:], wqk_sb[:, s, m * 128:(m + 1) * 128],
                                         xt_ch[:, s, :], start=(s == 0), stop=False)
                    nc.tensor.matmul(pqk[:], bqk_sb[0:1, m * 128:(m + 1) * 128],
                                     ones_r[0:1, :], start=False, stop=True,
                                     skip_group_check=True)
                    tQr = work.tile([128, TCH], BF16, tag="tQr")
                    nc.vector.tensor_copy(tQr[:], pqk[:])
                    tQc = work.tile([128, TCH], BF16, tag="tQc")
                    nc.vector.scalar_tensor_tensor(
                        tQc[:], tQr[:], 1.0, cos_sb[:, tcols], MUL, MUL)
                    tQs = work.tile([128, TCH], BF16, tag="tQs")
                    for (a0, a1, b0, b1) in swap:
                        nc.vector.scalar_tensor_tensor(
                            tQs[a0:a1, :], tQr[b0:b1, :], 1.0,
                            sin_sb[b0:b1, tcols], MUL, MUL)
                    dest = (qT if m % 2 == 0 else kT)[m // 2]
                    nc.vector.scalar_tensor_tensor(
                        dest[:, tcols], tQc[:], 1.0, tQs[:], MUL, ADD)

                # v matmuls (natural orientation) + eviction into v_sb
                for ts in range(4):
                    pv = ps_a.tile([128, 256], F32, tag="a")
                    for s in range(8):
                        nc.tensor.matmul(pv[:], xt_ch[:, s, ts * 128:(ts + 1) * 128],
                                         wv_sb[:, s, :], start=(s == 0), stop=False)
                    nc.tensor.matmul(pv[:], ones_r[0:1, 0:128], bv_sb[0:1, :],
                                     start=False, stop=True, skip_group_check=True)
                    sl = tcH * 4 + ts
                    nc.scalar.activation(
                        v_sb[:, sl, :, 0:64],
                        pv[:].rearrange("p (h e) -> p h e", e=64), AF.Copy)

                # ======== attention for q-chunk tcH ========
                yT_ch = yp.tile([128, 2, TCH], BF16, tag="yT")
                nslabs = 4 * tcH + 4

                for p in range(2):
                    pos = [ps_o.tile([128, TCH], F32, tag="O", name=f"po{hh}")
                           for hh in range(2)]

                    def emit_S(j):
                        rr = j - 4 * tcH
                        r = max(rr, 0) * 128
                        qs = slice(tcH * TCH + r, (tcH + 1) * TCH)
                        psS = ps_s.tile([128, 2, TCH], F32, tag="S")
                        for hh in range(2):
                            base = 64 * hh
                            nc.tensor.matmul(
                                psS[:, hh, r:TCH],
                                kT[p][base:base + 64, j * 128:(j + 1) * 128],
                                qT[p][base:base + 64, qs],
                                start=True, stop=True)
                        expS = ep.tile([128, 2, TCH], BF16, tag="expS")
                        nc.scalar.activation(expS[:, :, r:TCH], psS[:, :, r:TCH],
                                             AF.Exp, scale=0.125)
                        if rr >= 0:
                            for hh in range(2):
                                nc.gpsimd.tensor_mul(expS[:, hh, r:r + 128],
                                                     expS[:, hh, r:r + 128],
                                                     msk_sb[:, :])
                        return expS, r

                    def emit_PV(j, expS, r):
                        for hh in range(2):
                            h = 2 * p + hh
                            nc.tensor.matmul(pos[hh][0:65, r:TCH],
                                             v_sb[:, j, h, :],
                                             expS[:, hh, r:TCH],
                                             start=(j == 0),
                                             stop=(j == nslabs - 1))

                    # software pipeline: S(j+1) issued before PV(j) so the
                    # exp latency is hidden behind PE work
                    prev = emit_S(0)
                    for j in range(1, nslabs):
                        cur = emit_S(j)
                        emit_PV(j - 1, *prev)
                        prev = cur
                    emit_PV(nslabs - 1, *prev)

                    for hh in range(2):
                        base, po = 64 * hh, pos[hh]
                        l_r = work.tile([1, TCH], F32R, tag="lr")
                        with nc.allow_low_precision(reason="f32r rounding of 1/l"):
                            nc.vector.reciprocal(l_r[:], po[64:65, :])
                        lbc = work.tile([64, TCH], F32R, tag="lbc")
                        nc.gpsimd.partition_broadcast(lbc[:], l_r[:])
                        nc.vector.scalar_tensor_tensor(
                            yT_ch[base:base + 64, p, :], po[0:64, :], 1.0,
                            lbc[:], MUL, MUL)

                # ======== proj for q-chunk tcH ========
                for mp in range(4):
                    pp = ps_s.tile([128, 2, TCH], F32, tag="S")
                    for half in range(2):
                        mo = 2 * mp + half
                        for s in range(2):
                            nc.tensor.matmul(pp[:, half, :],
                                             wp_sb[:, s, mo * 128:(mo + 1) * 128],
                                             yT_ch[:, s, :], start=(s == 0),
                                             stop=(s == 1))
                    o_st = ost.tile([128, 2, TCH], BF16, tag="ost")
                    nc.vector.tensor_copy(o_st[:], pp[:])
                    for half in range(2):
                        mo = 2 * mp + half
                        nc.sync.dma_start(out_t[mo * 128:(mo + 1) * 128, tcols],
                                          o_st[:, half, :])

    nc.compile()
    return nc


def _rope_tables():
    pos = np.arange(T, dtype=np.float32)[:, None]                  # [T, 1]
    i = np.arange(1, HS // 2 + 1, dtype=np.float32)[None]          # [1, 32]
    theta = 1.0 / 10000.0 ** (2.0 * (i - 1.0) / HS)
    ang = pos * theta
    cos, sin = np.cos(ang).T, np.sin(ang).T                        # [32, T]
    cos_rep = np.tile(cos, (4, 1)).astype(np.float32)              # [128, T]
    sin_sgn = np.concatenate([sin, -sin, sin, -sin], 0).astype(np.float32)
    return cos_rep, sin_sgn


def _mask128():
    p = np.arange(128)[:, None]
    f = np.arange(128)[None, :]
    return (p <= f).astype(np.float32)


def kernel(x, W_qkv, b_qkv, W_proj, b_proj):
    global last_results
    import ml_dtypes
    from concourse.bass_utils import run_bass_kernel_spmd

    bf16 = ml_dtypes.bfloat16

    if "nc" not in _cache:
        _cache["nc"] = _build()
    nc = _cache["nc"]

    x = np.asarray(x, np.float32)
    W_qkv = np.asarray(W_qkv, np.float32)
    b_qkv = np.asarray(b_qkv, np.float32)
    W_proj = np.asarray(W_proj, np.float32)
    b_proj = np.asarray(b_proj, np.float32)

    perm = np.concatenate([np.arange(0, HS, 2), np.arange(1, HS, 2)])  # even|odd
    cos_rep, sin_sgn = _rope_tables()
    cmask = _mask128()

    in_maps = []
    for core in range(NCORES):
        b, g = core // 4, core % 4
        heads = [4 * g + j for j in range(HPC)]
        wq = [W_qkv[:, h * 3 * HS:h * 3 * HS + HS][:, perm] for h in heads]
        wk = [W_qkv[:, h * 3 * HS + HS:h * 3 * HS + 2 * HS][:, perm] for h in heads]
        wv_ = [W_qkv[:, h * 3 * HS + 2 * HS:h * 3 * HS + 3 * HS] for h in heads]
        bq = [b_qkv[h * 3 * HS:h * 3 * HS + HS][perm] for h in heads]
        bk = [b_qkv[h * 3 * HS + HS:h * 3 * HS + 2 * HS][perm] for h in heads]
        bv_ = [b_qkv[h * 3 * HS + 2 * HS:h * 3 * HS + 3 * HS] for h in heads]
        # col-chunks: [q01 | k01 | q23 | k23]
        wqk = np.concatenate([wq[0], wq[1], wk[0], wk[1],
                              wq[2], wq[3], wk[2], wk[3]], axis=1)
        bqk = np.concatenate([bq[0], bq[1], bk[0], bk[1],
                              bq[2], bq[3], bk[2], bk[3]])[None, :]
        in_maps.append({
            "xt_in": np.ascontiguousarray(x[b].T).astype(bf16),
            "wqk": np.ascontiguousarray(wqk).astype(bf16),
            "bqk": np.ascontiguousarray(bqk),
            "wv": np.ascontiguousarray(np.concatenate(wv_, axis=1)).astype(bf16),
            "bv": np.ascontiguousarray(np.concatenate(bv_)[None, :]),
            "wp": np.ascontiguousarray(W_proj[g * 256:(g + 1) * 256, :]).astype(bf16),
            "cos_in": cos_rep.astype(bf16),
            "sin_in": sin_sgn.astype(bf16),
            "cmask": cmask.astype(bf16),
        })

    res = run_bass_kernel_spmd(nc, in_maps, core_ids=list(range(NCORES)))
    last_results = res

    out = np.zeros((B, T, C), dtype=np.float32)
    for core in range(NCORES):
        b = core // 4
        out[b] += res.results[core]["out_t"].astype(np.float32).T
    out += b_proj[None, None, :]
    return out


# revision 19
# speedup vs baseline: 1.3750x; 1.0237x over previous
"""Causal self-attention (B=2, T=2048, C=1024, H=16, rope) on 8 trn2 cores.

Sharding: core i = (batch b = i // 4, head-group g = i % 4 owning heads 4g..4g+3).
Each core computes its 4 heads' attention and a partial projection (transposed,
bf16); the host sums the 4 head-group partials per batch and adds b_proj.

v2: all matmul operands bf16 (1 cycle/row at any N), x transposed on the host
(no PE transposes / DVE staging), rope as 6 scalar_tensor_tensor ops on DVE
(4x perf mode on all-SBUF bf16), qk psum evict on Pool, exp pairs on Act,
proj evict on Pool, bf16 output.

On-core dataflow:
  xT chunk  = DMA of host-transposed x          [128, 8, 512] bf16
  qkT = W_qk_perm.T @ x.T (+bias K=1 f32r mm)   [512, T] -> rope -> qT/kT bf16
  v   = x @ W_v  (+bias)                        [T, 256], + ones col, bf16
  S.T pair  = kT-slab.T @ qT-chunk (K=64)       [128k, 2, 512q] f32 psum
  expS = exp(S.T / 8) (Act, pair op) -> bf16, causal-masked on diag (Pool)
  [O.T; l] = sum_k [v | 1].T-slab @ expS        PV + denominators, f32 psum
  yT = O.T * (1/l)  (DVE recip + K=1 ones mm broadcast + DVE stt)
  outT = W_p-shard.T @ yT                       [1024, T] bf16 partial, DMA out
"""

import numpy as np

B, T, C, H = 2, 2048, 1024, 16
HS = C // H            # 64
HPC = H // 4           # 4 heads per core
NCORES = 8
TCH = 512              # t/q chunk size
NCH = T // TCH         # 4 chunks
NSLAB = T // 128       # 16 t-slabs

_cache = {}
last_results = None    # BassKernelResults of the most recent run (for test.py)


def _build():
    import concourse.bacc as bacc
    import concourse.mybir as mybir
    import concourse.tile as tile

    F32 = mybir.dt.float32
    F32R = mybir.dt.float32r
    BF16 = mybir.dt.bfloat16
    AF = mybir.ActivationFunctionType
    MUL = mybir.AluOpType.mult
    ADD = mybir.AluOpType.add

    nc = bacc.Bacc("TRN2", target_bir_lowering=False, debug=False,
                   num_devices=NCORES)

    xt_in = nc.dram_tensor("xt_in", (C, T), BF16, kind="ExternalInput")
    wqk = nc.dram_tensor("wqk", (C, 512), BF16, kind="ExternalInput")
    bqk = nc.dram_tensor("bqk", (1, 512), F32R, kind="ExternalInput")
    wv = nc.dram_tensor("wv", (C, 256), BF16, kind="ExternalInput")
    bv = nc.dram_tensor("bv", (1, 256), F32R, kind="ExternalInput")
    wp = nc.dram_tensor("wp", (256, C), BF16, kind="ExternalInput")
    cos_in = nc.dram_tensor("cos_in", (128, T), BF16, kind="ExternalInput")
    sin_in = nc.dram_tensor("sin_in", (128, T), BF16, kind="ExternalInput")
    cmask = nc.dram_tensor("cmask", (128, 128), BF16, kind="ExternalInput")
    out_t = nc.dram_tensor("out_t", (C, T), BF16, kind="ExternalOutput")

    with tile.TileContext(nc) as tc:
        with (
            tc.tile_pool(name="const", bufs=1) as const,
            tc.tile_pool(name="xp", bufs=2) as xp,
            tc.tile_pool(name="work", bufs=3) as work,
            tc.tile_pool(name="ep", bufs=4) as ep,
            tc.tile_pool(name="yp", bufs=2) as yp,
            tc.tile_pool(name="ost", bufs=2) as ost,
            tc.tile_pool(name="ps_a", bufs=2, space="PSUM") as ps_a,
            tc.tile_pool(name="ps_s", bufs=2, space="PSUM") as ps_s,
            tc.tile_pool(name="ps_o", bufs=2, space="PSUM") as ps_o,
        ):
            # ---- x chunk 0 + weights first (DMA priority), rest behind ----
            xts = [xp.tile([128, 8, TCH], BF16, tag="xt", name=f"xt{c}")
                   for c in range(NCH)]
            nc.sync.dma_start(
                xts[0][:],
                xt_in.ap()[:, 0:TCH].rearrange("(s p) m -> p s m", p=128))
            wqk_sb = const.tile([128, 8, 512], BF16)
            nc.scalar.dma_start(wqk_sb[:],
                                wqk.ap().rearrange("(s p) m -> p s m", p=128))
            bqk_sb = const.tile([1, 512], F32R)
            nc.gpsimd.dma_start(bqk_sb[:], bqk[:, :])
            cos_sb = const.tile([128, T], BF16)
            nc.gpsimd.dma_start(cos_sb[:], cos_in[:, :])
            sin_sb = const.tile([128, T], BF16)
            nc.gpsimd.dma_start(sin_sb[:], sin_in[:, :])
            wv_sb = const.tile([128, 8, 256], BF16)
            nc.sync.dma_start(wv_sb[:],
                              wv.ap().rearrange("(s p) m -> p s m", p=128))
            bv_sb = const.tile([1, 256], F32R)
            nc.gpsimd.dma_start(bv_sb[:], bv[:, :])
            msk_sb = const.tile([128, 128], BF16)
            nc.gpsimd.dma_start(msk_sb[:], cmask[:, :])
            wp_sb = const.tile([128, 2, C], BF16)
            nc.scalar.dma_start(wp_sb[:],
                                wp.ap().rearrange("(s p) m -> p s m", p=128))

            ones_f = const.tile([1, TCH], F32)
            nc.gpsimd.memset(ones_f[:], 1.0)
            ones_r = const.tile([1, TCH], F32R)
            nc.vector.tensor_copy(ones_r[:], ones_f[:])
            onec_f = const.tile([128, 1], F32)
            nc.gpsimd.memset(onec_f[:], 1.0)
            onec = const.tile([128, 1], BF16)
            nc.vector.tensor_copy(onec[:], onec_f[:])

            # ---- persistent activations ----
            qT = [const.tile([128, T], BF16, name=f"qT{p}", tag=f"qT{p}")
                  for p in range(2)]
            kT = [const.tile([128, T], BF16, name=f"kT{p}", tag=f"kT{p}")
                  for p in range(2)]
            # v with ones column: [t-slab-part, slab, head, 65]
            v_sb = const.tile([128, NSLAB, HPC, 65], BF16)
            ones128 = const.tile([128, 64], F32)
            nc.gpsimd.memset(ones128[:], 1.0)
            nc.vector.tensor_copy(
                v_sb[:, :, :, 64],
                ones128[:, 0:64].rearrange("p (s h) -> p s h", s=NSLAB))

            swap = [(0, 32, 32, 64), (32, 64, 0, 32),
                    (64, 96, 96, 128), (96, 128, 64, 96)]

            for tcH in range(NCH):
                tcols = slice(tcH * TCH, (tcH + 1) * TCH)
                # ======== phase A: xT chunk prefetch, qk(+rope), v ========
                xt_ch = xts[tcH]
                if tcH + 1 < NCH:
                    nxt = slice((tcH + 1) * TCH, (tcH + 2) * TCH)
                    nc.sync.dma_start(
                        xts[tcH + 1][:],
                        xt_in.ap()[:, nxt].rearrange("(s p) m -> p s m", p=128))

                # qk matmuls + rope eviction
                for m in range(4):
                    pqk = ps_a.tile([128, TCH], F32, tag="a")
                    for s in range(8):
                        nc.tensor.matmul(pqk[:], wqk_sb[:, s, m * 128:(m + 1) * 128],
                                         xt_ch[:, s, :], start=(s == 0), stop=False)
                    nc.tensor.matmul(pqk[:], bqk_sb[0:1, m * 128:(m + 1) * 128],
                                     ones_r[0:1, :], start=False, stop=True,
                                     skip_group_check=True)
                    tQr = work.tile([128, TCH], BF16, tag="tQr")
                    nc.vector.tensor_copy(tQr[:], pqk[:])
                    tQc = work.tile([128, TCH], BF16, tag="tQc")
                    nc.vector.scalar_tensor_tensor(
                        tQc[:], tQr[:], onec[:, 0:1], cos_sb[:, tcols], MUL, MUL)
                    tQs = work.tile([128, TCH], BF16, tag="tQs")
                    for (a0, a1, b0, b1) in swap:
                        nc.vector.scalar_tensor_tensor(
                            tQs[a0:a1, :], tQr[b0:b1, :], onec[b0:b1, 0:1],
                            sin_sb[b0:b1, tcols], MUL, MUL)
                    dest = (qT if m % 2 == 0 else kT)[m // 2]
                    nc.vector.scalar_tensor_tensor(
                        dest[:, tcols], tQc[:], onec[:, 0:1], tQs[:], MUL, ADD)

                # v matmuls (natural orientation) + eviction into v_sb
                for ts in range(4):
                    pv = ps_a.tile([128, 256], F32, tag="a")
                    for s in range(8):
                        nc.tensor.matmul(pv[:], xt_ch[:, s, ts * 128:(ts + 1) * 128],
                                         wv_sb[:, s, :], start=(s == 0), stop=False)
                    nc.tensor.matmul(pv[:], ones_r[0:1, 0:128], bv_sb[0:1, :],
                                     start=False, stop=True, skip_group_check=True)
                    sl = tcH * 4 + ts
                    nc.scalar.activation(
                        v_sb[:, sl, :, 0:64],
                        pv[:].rearrange("p (h e) -> p h e", e=64), AF.Copy)

                # ======== attention for q-chunk tcH ========
                yT_ch = yp.tile([128, 2, TCH], BF16, tag="yT")
                nslabs = 4 * tcH + 4

                for p in range(2):
                    pos = [ps_o.tile([128, TCH], F32, tag="O", name=f"po{hh}")
                           for hh in range(2)]

                    def emit_S(j):
                        rr = j - 4 * tcH
                        r = max(rr, 0) * 128
                        qs = slice(tcH * TCH + r, (tcH + 1) * TCH)
                        psS = ps_s.tile([128, 2, TCH], F32, tag="S")
                        for hh in range(2):
                            base = 64 * hh
                            nc.tensor.matmul(
                                psS[:, hh, r:TCH],
                                kT[p][base:base + 64, j * 128:(j + 1) * 128],
                                qT[p][base:base + 64, qs],
                                start=True, stop=True)
                        expS = ep.tile([128, 2, TCH], BF16, tag="expS")
                        nc.scalar.activation(expS[:, :, r:TCH], psS[:, :, r:TCH],
                                             AF.Exp, scale=0.125)
                        if rr >= 0:
                            for hh in range(2):
                                nc.vector.scalar_tensor_tensor(
                                    expS[:, hh, r:r + 128],
                                    expS[:, hh, r:r + 128],
                                    onec[:, 0:1], msk_sb[:, :], MUL, MUL)
                        return expS, r

                    def emit_PV(j, expS, r):
                        for hh in range(2):
                            h = 2 * p + hh
                            nc.tensor.matmul(pos[hh][0:65, r:TCH],
                                             v_sb[:, j, h, :],
                                             expS[:, hh, r:TCH],
                                             start=(j == 0),
                                             stop=(j == nslabs - 1))

                    # software pipeline: S(j+1) issued before PV(j) so the
                    # exp latency is hidden behind PE work
                    prev = emit_S(0)
                    for j in range(1, nslabs):
                        cur = emit_S(j)
                        emit_PV(j - 1, *prev)
                        prev = cur
                    emit_PV(nslabs - 1, *prev)

                    for hh in range(2):
                        base, po = 64 * hh, pos[hh]
                        l_r = work.tile([1, TCH], F32R, tag="lr")
                        with nc.allow_low_precision(reason="f32r rounding of 1/l"):
                            nc.vector.reciprocal(l_r[:], po[64:65, :])
                        lbc = work.tile([64, TCH], F32R, tag="lbc")
                        nc.gpsimd.partition_broadcast(lbc[:], l_r[:])
                        nc.vector.scalar_tensor_tensor(
                            yT_ch[base:base + 64, p, :], po[0:64, :],
                            onec[0:64, 0:1], lbc[:], MUL, MUL)

                # ======== proj for q-chunk tcH ========
                for mp in range(4):
                    pp = ps_s.tile([128, 2, TCH], F32, tag="S")
                    for half in range(2):
                        mo = 2 * mp + half
                        for s in range(2):
                            nc.tensor.matmul(pp[:, half, :],
                                             wp_sb[:, s, mo * 128:(mo + 1) * 128],
                                             yT_ch[:, s, :], start=(s == 0),
                                             stop=(s == 1))
                    o_st = ost.tile([128, 2, TCH], BF16, tag="ost")
                    nc.vector.tensor_copy(o_st[:], pp[:])
                    for half in range(2):
                        mo = 2 * mp + half
                        nc.sync.dma_start(out_t[mo * 128:(mo + 1) * 128, tcols],
                                          o_st[:, half, :])

    nc.compile()
    return nc


def _rope_tables():
    pos = np.arange(T, dtype=np.float32)[:, None]                  # [T, 1]
    i = np.arange(1, HS // 2 + 1, dtype=np.float32)[None]          # [1, 32]
    theta = 1.0 / 10000.0 ** (2.0 * (i - 1.0) / HS)
    ang = pos * theta
    cos, sin = np.cos(ang).T, np.sin(ang).T                        # [32, T]
    cos_rep = np.tile(cos, (4, 1)).astype(np.float32)              # [128, T]
    sin_sgn = np.concatenate([sin, -sin, sin, -sin], 0).astype(np.float32)
    return cos_rep, sin_sgn


def _mask128():
    p = np.arange(128)[:, None]
    f = np.arange(128)[None, :]
    return (p <= f).astype(np.float32)


def kernel(x, W_qkv, b_qkv, W_proj, b_proj):
    global last_results
    import ml_dtypes
    from concourse.bass_utils import run_bass_kernel_spmd

    bf16 = ml_dtypes.bfloat16

    if "nc" not in _cache:
        _cache["nc"] = _build()
    nc = _cache["nc"]

    x = np.asarray(x, np.float32)
    W_qkv = np.asarray(W_qkv, np.float32)
    b_qkv = np.asarray(b_qkv, np.float32)
    W_proj = np.asarray(W_proj, np.float32)
    b_proj = np.asarray(b_proj, np.float32)

    perm = np.concatenate([np.arange(0, HS, 2), np.arange(1, HS, 2)])  # even|odd
    cos_rep, sin_sgn = _rope_tables()
    cmask = _mask128()

    in_maps = []
    for core in range(NCORES):
        b, g = core // 4, core % 4
        heads = [4 * g + j for j in range(HPC)]
        wq = [W_qkv[:, h * 3 * HS:h * 3 * HS + HS][:, perm] for h in heads]
        wk = [W_qkv[:, h * 3 * HS + HS:h * 3 * HS + 2 * HS][:, perm] for h in heads]
        wv_ = [W_qkv[:, h * 3 * HS + 2 * HS:h * 3 * HS + 3 * HS] for h in heads]
        bq = [b_qkv[h * 3 * HS:h * 3 * HS + HS][perm] for h in heads]
        bk = [b_qkv[h * 3 * HS + HS:h * 3 * HS + 2 * HS][perm] for h in heads]
        bv_ = [b_qkv[h * 3 * HS + 2 * HS:h * 3 * HS + 3 * HS] for h in heads]
        # col-chunks: [q01 | k01 | q23 | k23]
        wqk = np.concatenate([wq[0], wq[1], wk[0], wk[1],
                              wq[2], wq[3], wk[2], wk[3]], axis=1)
        bqk = np.concatenate([bq[0], bq[1], bk[0], bk[1],
                              bq[2], bq[3], bk[2], bk[3]])[None, :]
        in_maps.append({
            "xt_in": np.ascontiguousarray(x[b].T).astype(bf16),
            "wqk": np.ascontiguousarray(wqk).astype(bf16),
            "bqk": np.ascontiguousarray(bqk),
            "wv": np.ascontiguousarray(np.concatenate(wv_, axis=1)).astype(bf16),
            "bv": np.ascontiguousarray(np.concatenate(bv_)[None, :]),
            "wp": np.ascontiguousarray(W_proj[g * 256:(g + 1) * 256, :]).astype(bf16),
            "cos_in": cos_rep.astype(bf16),
            "sin_in": sin_sgn.astype(bf16),
            "cmask": cmask.astype(bf16),
        })

    res = run_bass_kernel_spmd(nc, in_maps, core_ids=list(range(NCORES)))
    last_results = res

    out = np.zeros((B, T, C), dtype=np.float32)
    for core in range(NCORES):
        b = core // 4
        out[b] += res.results[core]["out_t"].astype(np.float32).T
    out += b_proj[None, None, :]
    return out


# revision 26
# speedup vs baseline: 1.4383x; 1.0460x over previous
"""Causal self-attention (B=2, T=2048, C=1024, H=16, rope) on 8 trn2 cores.

Sharding: core i = (batch b = i // 4, head-group g = i % 4 owning heads 4g..4g+3).
Each core computes its 4 heads' attention and a partial projection (transposed,
bf16); the host sums the 4 head-group partials per batch and adds b_proj.

v2: all matmul operands bf16 (1 cycle/row at any N), x transposed on the host
(no PE transposes / DVE staging), rope as 6 scalar_tensor_tensor ops on DVE
(4x perf mode on all-SBUF bf16), qk psum evict on Pool, exp pairs on Act,
proj evict on Pool, bf16 output.

On-core dataflow:
  xT chunk  = DMA of host-transposed x          [128, 8, 512] bf16
  qkT = W_qk_perm.T @ x.T (+bias K=1 f32r mm)   [512, T] -> rope -> qT/kT bf16
  v   = x @ W_v  (+bias)                        [T, 256], + ones col, bf16
  S.T pair  = kT-slab.T @ qT-chunk (K=64)       [128k, 2, 512q] f32 psum
  expS = exp(S.T / 8) (Act, pair op) -> bf16, causal-masked on diag (Pool)
  [O.T; l] = sum_k [v | 1].T-slab @ expS        PV + denominators, f32 psum
  yT = O.T * (1/l)  (DVE recip + K=1 ones mm broadcast + DVE stt)
  outT = W_p-shard.T @ yT                       [1024, T] bf16 partial, DMA out
"""

import numpy as np

B, T, C, H = 2, 2048, 1024, 16
HS = C // H            # 64
HPC = H // 4           # 4 heads per core
NCORES = 8
TCH = 512              # t/q chunk size
NCH = T // TCH         # 4 chunks
NSLAB = T // 128       # 16 t-slabs

_cache = {}
last_results = None    # BassKernelResults of the most recent run (for test.py)


def _build():
    import concourse.bacc as bacc
    import concourse.mybir as mybir
    import concourse.tile as tile

    F32 = mybir.dt.float32
    F32R = mybir.dt.float32r
    BF16 = mybir.dt.bfloat16
    AF = mybir.ActivationFunctionType
    MUL = mybir.AluOpType.mult
    ADD = mybir.AluOpType.add

    nc = bacc.Bacc("TRN2", target_bir_lowering=False, debug=False,
                   num_devices=NCORES)

    xt_in = nc.dram_tensor("xt_in", (C, T), BF16, kind="ExternalInput")
    wqk = nc.dram_tensor("wqk", (C, 512), BF16, kind="ExternalInput")
    bqk_c = nc.dram_tensor("bqk_c", (128, 4), F32, kind="ExternalInput")
    wv = nc.dram_tensor("wv", (C, 256), BF16, kind="ExternalInput")
    wp = nc.dram_tensor("wp", (256, C), BF16, kind="ExternalInput")
    cos_in = nc.dram_tensor("cos_in", (128, T), BF16, kind="ExternalInput")
    sin_in = nc.dram_tensor("sin_in", (128, T), BF16, kind="ExternalInput")
    cmask = nc.dram_tensor("cmask", (128, 128), BF16, kind="ExternalInput")
    out_t = nc.dram_tensor("out_t", (C, T), BF16, kind="ExternalOutput")

    with tile.TileContext(nc) as tc:
        with (
            tc.tile_pool(name="const", bufs=1) as const,
            tc.tile_pool(name="xp", bufs=2) as xp,
            tc.tile_pool(name="work", bufs=3) as work,
            tc.tile_pool(name="ep", bufs=4) as ep,
            tc.tile_pool(name="yp", bufs=2) as yp,
            tc.tile_pool(name="ost", bufs=2) as ost,
            tc.tile_pool(name="ps_a", bufs=2, space="PSUM") as ps_a,
            tc.tile_pool(name="ps_s", bufs=2, space="PSUM") as ps_s,
            tc.tile_pool(name="ps_o", bufs=2, space="PSUM") as ps_o,
        ):
            # ---- x chunk 0 + weights first (DMA priority, 3 queues), rest behind
            xts = [xp.tile([128, 8, TCH], BF16, tag="xt", name=f"xt{c}")
                   for c in range(NCH)]
            nc.sync.dma_start(
                xts[0][:, 0:4, :],
                xt_in.ap()[0:512, 0:TCH].rearrange("(s p) m -> p s m", p=128))
            nc.gpsimd.dma_start(
                xts[0][:, 4:8, :],
                xt_in.ap()[512:1024, 0:TCH].rearrange("(s p) m -> p s m", p=128))
            wqk_sb = const.tile([128, 8, 512], BF16)
            nc.scalar.dma_start(wqk_sb[:],
                                wqk.ap().rearrange("(s p) m -> p s m", p=128))
            bqk_sb = const.tile([128, 4], F32)
            nc.gpsimd.dma_start(bqk_sb[:], bqk_c[:, :])
            cos_sb = const.tile([128, T], BF16)
            nc.gpsimd.dma_start(cos_sb[:], cos_in[:, :])
            sin_sb = const.tile([128, T], BF16)
            nc.gpsimd.dma_start(sin_sb[:], sin_in[:, :])
            wv_sb = const.tile([128, 8, 256], BF16)
            nc.sync.dma_start(wv_sb[:],
                              wv.ap().rearrange("(s p) m -> p s m", p=128))
            msk_sb = const.tile([128, 128], BF16)
            nc.gpsimd.dma_start(msk_sb[:], cmask[:, :])
            wp_sb = const.tile([128, 2, C], BF16)
            nc.scalar.dma_start(wp_sb[:],
                                wp.ap().rearrange("(s p) m -> p s m", p=128))

            # ---- persistent activations ----
            qT = [const.tile([128, T], BF16, name=f"qT{p}", tag=f"qT{p}")
                  for p in range(2)]
            kT = [const.tile([128, T], BF16, name=f"kT{p}", tag=f"kT{p}")
                  for p in range(2)]
            # v with ones column: [t-slab-part, slab, head, 65]
            v_sb = const.tile([128, NSLAB, HPC, 65], BF16)
            ones128 = const.tile([128, 64], F32)
            nc.gpsimd.memset(ones128[:], 1.0)
            nc.vector.tensor_copy(
                v_sb[:, :, :, 64],
                ones128[:, 0:64].rearrange("p (s h) -> p s h", s=NSLAB))

            swap = [(0, 32, 32, 64), (32, 64, 0, 32),
                    (64, 96, 96, 128), (96, 128, 64, 96)]

            for tcH in range(NCH):
                tcols = slice(tcH * TCH, (tcH + 1) * TCH)
                # ======== phase A: xT chunk prefetch, qk(+rope), v ========
                xt_ch = xts[tcH]
                if tcH + 1 < NCH:
                    nxt = slice((tcH + 1) * TCH, (tcH + 2) * TCH)
                    nc.sync.dma_start(
                        xts[tcH + 1][:],
                        xt_in.ap()[:, nxt].rearrange("(s p) m -> p s m", p=128))

                # qk matmuls; bias folded into the DVE evict (per-partition add)
                for m in range(4):
                    pqk = ps_a.tile([128, TCH], F32, tag="a")
                    for s in range(8):
                        nc.tensor.matmul(pqk[:], wqk_sb[:, s, m * 128:(m + 1) * 128],
                                         xt_ch[:, s, :], start=(s == 0),
                                         stop=(s == 7))
                    tQr = work.tile([128, TCH], BF16, tag="tQr")
                    nc.vector.tensor_scalar_add(tQr[:], pqk[:],
                                                bqk_sb[:, m:m + 1])
                    tQc = work.tile([128, TCH], BF16, tag="tQc")
                    nc.vector.tensor_mul(tQc[:], tQr[:], cos_sb[:, tcols])
                    tQs = work.tile([128, TCH], BF16, tag="tQs")
                    for (a0, a1, b0, b1) in swap:
                        nc.vector.tensor_mul(tQs[a0:a1, :], tQr[b0:b1, :],
                                             sin_sb[b0:b1, tcols])
                    dest = (qT if m % 2 == 0 else kT)[m // 2]
                    nc.vector.tensor_add(dest[:, tcols], tQc[:], tQs[:])

                # v matmuls (natural orientation; bias folded into host-side
                # output correction since sum(att) == 1) + evict into v_sb
                for ts in range(4):
                    pv = ps_a.tile([128, 256], F32, tag="a")
                    for s in range(8):
                        nc.tensor.matmul(pv[:], xt_ch[:, s, ts * 128:(ts + 1) * 128],
                                         wv_sb[:, s, :], start=(s == 0),
                                         stop=(s == 7))
                    sl = tcH * 4 + ts
                    nc.scalar.activation(
                        v_sb[:, sl, :, 0:64],
                        pv[:].rearrange("p (h e) -> p h e", e=64), AF.Copy)

                # ======== attention for q-chunk tcH ========
                yT_ch = yp.tile([128, 2, TCH], BF16, tag="yT")
                nslabs = 4 * tcH + 4

                for p in range(2):
                    pos = [ps_o.tile([128, TCH], F32, tag="O", name=f"po{hh}")
                           for hh in range(2)]

                    def emit_S(j):
                        rr = j - 4 * tcH
                        r = max(rr, 0) * 128
                        qs = slice(tcH * TCH + r, (tcH + 1) * TCH)
                        psS = ps_s.tile([128, 2, TCH], F32, tag="S")
                        for hh in range(2):
                            base = 64 * hh
                            nc.tensor.matmul(
                                psS[:, hh, r:TCH],
                                kT[p][base:base + 64, j * 128:(j + 1) * 128],
                                qT[p][base:base + 64, qs],
                                start=True, stop=True)
                        expS = ep.tile([128, 2, TCH], BF16, tag="expS")
                        nc.scalar.activation(expS[:, :, r:TCH], psS[:, :, r:TCH],
                                             AF.Exp, scale=0.125)
                        if rr >= 0:
                            for hh in range(2):
                                nc.vector.tensor_mul(expS[:, hh, r:r + 128],
                                                     expS[:, hh, r:r + 128],
                                                     msk_sb[:, :])
                        return expS, r

                    def emit_PV(j, expS, r, hh):
                        h = 2 * p + hh
                        nc.tensor.matmul(pos[hh][0:65, r:TCH],
                                         v_sb[:, j, h, :],
                                         expS[:, hh, r:TCH],
                                         start=(j == 0),
                                         stop=(j == nslabs - 1))

                    def emit_norm(hh):
                        base, po = 64 * hh, pos[hh]
                        l_r = work.tile([1, TCH], F32R, tag="lr")
                        with nc.allow_low_precision(reason="f32r rounding of 1/l"):
                            nc.vector.reciprocal(l_r[:], po[64:65, :])
                        lbc = work.tile([64, TCH], F32R, tag="lbc")
                        nc.gpsimd.partition_broadcast(lbc[:], l_r[:])
                        nc.vector.tensor_mul(yT_ch[base:base + 64, p, :],
                                             po[0:64, :], lbc[:])

                    # software pipeline: S(j+1) issued before PV(j) so the
                    # exp latency is hidden behind PE work
                    prev = emit_S(0)
                    for j in range(1, nslabs):
                        cur = emit_S(j)
                        emit_PV(j - 1, prev[0], prev[1], 0)
                        emit_PV(j - 1, prev[0], prev[1], 1)
                        prev = cur
                    # last slab: finish each head's PV then immediately start
                    # its normalize chain so it overlaps the other head's PV
                    emit_PV(nslabs - 1, prev[0], prev[1], 0)
                    emit_norm(0)
                    emit_PV(nslabs - 1, prev[0], prev[1], 1)
                    emit_norm(1)

                # ======== proj for q-chunk tcH ========
                for mp in range(4):
                    pp = ps_s.tile([128, 2, TCH], F32, tag="S")
                    for half in range(2):
                        mo = 2 * mp + half
                        for s in range(2):
                            nc.tensor.matmul(pp[:, half, :],
                                             wp_sb[:, s, mo * 128:(mo + 1) * 128],
                                             yT_ch[:, s, :], start=(s == 0),
                                             stop=(s == 1))
                    o_st = ost.tile([128, 2, TCH], BF16, tag="ost")
                    nc.vector.tensor_copy(o_st[:], pp[:])
                    for half in range(2):
                        mo = 2 * mp + half
                        nc.sync.dma_start(out_t[mo * 128:(mo + 1) * 128, tcols],
                                          o_st[:, half, :])

    nc.compile()
    return nc


def _rope_tables():
    pos = np.arange(T, dtype=np.float32)[:, None]                  # [T, 1]
    i = np.arange(1, HS // 2 + 1, dtype=np.float32)[None]          # [1, 32]
    theta = 1.0 / 10000.0 ** (2.0 * (i - 1.0) / HS)
    ang = pos * theta
    cos, sin = np.cos(ang).T, np.sin(ang).T                        # [32, T]
    cos_rep = np.tile(cos, (4, 1)).astype(np.float32)              # [128, T]
    sin_sgn = np.concatenate([sin, -sin, sin, -sin], 0).astype(np.float32)
    return cos_rep, sin_sgn


def _mask128():
    p = np.arange(128)[:, None]
    f = np.arange(128)[None, :]
    return (p <= f).astype(np.float32)


def kernel(x, W_qkv, b_qkv, W_proj, b_proj):
    global last_results
    import ml_dtypes
    from concourse.bass_utils import run_bass_kernel_spmd

    bf16 = ml_dtypes.bfloat16

    if "nc" not in _cache:
        _cache["nc"] = _build()
    nc = _cache["nc"]

    x = np.asarray(x, np.float32)
    W_qkv = np.asarray(W_qkv, np.float32)
    b_qkv = np.asarray(b_qkv, np.float32)
    W_proj = np.asarray(W_proj, np.float32)
    b_proj = np.asarray(b_proj, np.float32)

    perm = np.concatenate([np.arange(0, HS, 2), np.arange(1, HS, 2)])  # even|odd
    cos_rep, sin_sgn = _rope_tables()
    cmask = _mask128()

    in_maps = []
    for core in range(NCORES):
        b, g = core // 4, core % 4
        heads = [4 * g + j for j in range(HPC)]
        wq = [W_qkv[:, h * 3 * HS:h * 3 * HS + HS][:, perm] for h in heads]
        wk = [W_qkv[:, h * 3 * HS + HS:h * 3 * HS + 2 * HS][:, perm] for h in heads]
        wv_ = [W_qkv[:, h * 3 * HS + 2 * HS:h * 3 * HS + 3 * HS] for h in heads]
        bq = [b_qkv[h * 3 * HS:h * 3 * HS + HS][perm] for h in heads]
        bk = [b_qkv[h * 3 * HS + HS:h * 3 * HS + 2 * HS][perm] for h in heads]
        bv_ = [b_qkv[h * 3 * HS + 2 * HS:h * 3 * HS + 3 * HS] for h in heads]
        # col-chunks: [q01 | k01 | q23 | k23]
        wqk = np.concatenate([wq[0], wq[1], wk[0], wk[1],
                              wq[2], wq[3], wk[2], wk[3]], axis=1)
        bqk = np.concatenate([bq[0], bq[1], bk[0], bk[1],
                              bq[2], bq[3], bk[2], bk[3]])
        in_maps.append({
            "xt_in": np.ascontiguousarray(x[b].T).astype(bf16),
            "wqk": np.ascontiguousarray(wqk).astype(bf16),
            "bqk_c": np.ascontiguousarray(bqk.reshape(4, 128).T),
            "wv": np.ascontiguousarray(np.concatenate(wv_, axis=1)).astype(bf16),
            "wp": np.ascontiguousarray(W_proj[g * 256:(g + 1) * 256, :]).astype(bf16),
            "cos_in": cos_rep.astype(bf16),
            "sin_in": sin_sgn.astype(bf16),
            "cmask": cmask.astype(bf16),
        })

    res = run_bass_kernel_spmd(nc, in_maps, core_ids=list(range(NCORES)))
    last_results = res

    out = np.zeros((B, T, C), dtype=np.float32)
    for core in range(NCORES):
        b = core // 4
        out[b] += res.results[core]["out_t"].astype(np.float32).T
    # v-bias shifts y by exactly bv per head (sum(att) == 1), so its effect
    # on the output is the constant bv_full @ W_proj
    bv_full = np.concatenate(
        [b_qkv[h * 3 * HS + 2 * HS:h * 3 * HS + 3 * HS] for h in range(H)])
    out += (b_proj + bv_full @ W_proj)[None, None, :]
    return out


# revision 30
# speedup vs baseline: 1.5070x; 1.0478x over previous
"""Causal self-attention (B=2, T=2048, C=1024, H=16, rope) on 8 trn2 cores.

Sharding: core i = (batch b = i // 4, head-group g = i % 4 owning heads 4g..4g+3).
Each core computes its 4 heads' attention and a partial projection (transposed,
bf16); the host sums the 4 head-group partials per batch and adds b_proj.

v2: all matmul operands bf16 (1 cycle/row at any N), x transposed on the host
(no PE transposes / DVE staging), rope as 6 scalar_tensor_tensor ops on DVE
(4x perf mode on all-SBUF bf16), qk psum evict on Pool, exp pairs on Act,
proj evict on Pool, bf16 output.

On-core dataflow:
  xT chunk  = DMA of host-transposed x          [128, 8, 512] bf16
  qkT = W_qk_perm.T @ x.T (+bias K=1 f32r mm)   [512, T] -> rope -> qT/kT bf16
  v   = x @ W_v  (+bias)                        [T, 256], + ones col, bf16
  S.T pair  = kT-slab.T @ qT-chunk (K=64)       [128k, 2, 512q] f32 psum
  expS = exp(S.T / 8) (Act, pair op) -> bf16, causal-masked on diag (Pool)
  [O.T; l] = sum_k [v | 1].T-slab @ expS        PV + denominators, f32 psum
  yT = O.T * (1/l)  (DVE recip + K=1 ones mm broadcast + DVE stt)
  outT = W_p-shard.T @ yT                       [1024, T] bf16 partial, DMA out
"""

import numpy as np

B, T, C, H = 2, 2048, 1024, 16
HS = C // H            # 64
HPC = H // 4           # 4 heads per core
NCORES = 8
TCH = 512              # t/q chunk size
NCH = T // TCH         # 4 chunks
NSLAB = T // 128       # 16 t-slabs

_cache = {}
last_results = None    # BassKernelResults of the most recent run (for test.py)


def _build():
    import concourse.bacc as bacc
    import concourse.mybir as mybir
    import concourse.tile as tile

    F32 = mybir.dt.float32
    F32R = mybir.dt.float32r
    BF16 = mybir.dt.bfloat16
    AF = mybir.ActivationFunctionType
    MUL = mybir.AluOpType.mult
    ADD = mybir.AluOpType.add

    nc = bacc.Bacc("TRN2", target_bir_lowering=False, debug=False,
                   num_devices=NCORES)

    xt_in = nc.dram_tensor("xt_in", (C, T), BF16, kind="ExternalInput")
    wqk = nc.dram_tensor("wqk", (C, 512), BF16, kind="ExternalInput")
    bqk_c = nc.dram_tensor("bqk_c", (128, 4), F32, kind="ExternalInput")
    wv = nc.dram_tensor("wv", (C, 256), BF16, kind="ExternalInput")
    wp = nc.dram_tensor("wp", (256, C), BF16, kind="ExternalInput")
    cos_in = nc.dram_tensor("cos_in", (128, T), BF16, kind="ExternalInput")
    sin_in = nc.dram_tensor("sin_in", (128, T), BF16, kind="ExternalInput")
    cmask = nc.dram_tensor("cmask", (128, 128), BF16, kind="ExternalInput")
    out_t = nc.dram_tensor("out_t", (C, T), BF16, kind="ExternalOutput")

    with tile.TileContext(nc) as tc:
        with (
            tc.tile_pool(name="const", bufs=1) as const,
            tc.tile_pool(name="xp", bufs=2) as xp,
            tc.tile_pool(name="work", bufs=3) as work,
            tc.tile_pool(name="ep", bufs=4) as ep,
            tc.tile_pool(name="yp", bufs=2) as yp,
            tc.tile_pool(name="ost", bufs=2) as ost,
            tc.tile_pool(name="ps_a", bufs=2, space="PSUM") as ps_a,
            tc.tile_pool(name="ps_s", bufs=2, space="PSUM") as ps_s,
            tc.tile_pool(name="ps_o", bufs=2, space="PSUM") as ps_o,
        ):
            # ---- x chunk 0 + weights first (DMA priority, 3 queues), rest behind
            xts = [xp.tile([128, 8, TCH], BF16, tag="xt", name=f"xt{c}")
                   for c in range(NCH)]
            nc.sync.dma_start(
                xts[0][:, 0:4, :],
                xt_in.ap()[0:512, 0:TCH].rearrange("(s p) m -> p s m", p=128))
            nc.gpsimd.dma_start(
                xts[0][:, 4:8, :],
                xt_in.ap()[512:1024, 0:TCH].rearrange("(s p) m -> p s m", p=128))
            # m-tile-major so the first qk matmul only waits on one small DMA
            wqk_sb = const.tile([128, 4, 8, 128], BF16)
            for m in range(4):
                nc.scalar.dma_start(
                    wqk_sb[:, m, :, :],
                    wqk.ap()[:, m * 128:(m + 1) * 128]
                    .rearrange("(s p) c -> p s c", p=128))
            bqk_sb = const.tile([128, 4], F32)
            nc.gpsimd.dma_start(bqk_sb[:], bqk_c[:, :])
            cos_sb = const.tile([128, T], BF16)
            nc.gpsimd.dma_start(cos_sb[:], cos_in[:, :])
            sin_sb = const.tile([128, T], BF16)
            nc.gpsimd.dma_start(sin_sb[:], sin_in[:, :])
            wv_sb = const.tile([128, 8, 256], BF16)
            nc.sync.dma_start(wv_sb[:],
                              wv.ap().rearrange("(s p) m -> p s m", p=128))
            msk_sb = const.tile([128, 128], BF16)
            nc.gpsimd.dma_start(msk_sb[:], cmask[:, :])
            wp_sb = const.tile([128, 2, C], BF16)
            nc.scalar.dma_start(wp_sb[:],
                                wp.ap().rearrange("(s p) m -> p s m", p=128))

            # ---- persistent activations ----
            qT = [const.tile([128, T], BF16, name=f"qT{p}", tag=f"qT{p}")
                  for p in range(2)]
            kT = [const.tile([128, T], BF16, name=f"kT{p}", tag=f"kT{p}")
                  for p in range(2)]
            # v with ones column: [t-slab-part, slab, head, 65]
            v_sb = const.tile([128, NSLAB, HPC, 65], BF16)
            ones128 = const.tile([128, 64], F32)
            nc.gpsimd.memset(ones128[:], 1.0)
            nc.vector.tensor_copy(
                v_sb[:, :, :, 64],
                ones128[:, 0:64].rearrange("p (s h) -> p s h", s=NSLAB))

            swap = [(0, 32, 32, 64), (32, 64, 0, 32),
                    (64, 96, 96, 128), (96, 128, 64, 96)]

            for tcH in range(NCH):
                tcols = slice(tcH * TCH, (tcH + 1) * TCH)
                # ======== phase A: xT chunk prefetch, qk(+rope), v ========
                xt_ch = xts[tcH]
                if tcH + 1 < NCH:
                    nxt = slice((tcH + 1) * TCH, (tcH + 2) * TCH)
                    nc.sync.dma_start(
                        xts[tcH + 1][:],
                        xt_in.ap()[:, nxt].rearrange("(s p) m -> p s m", p=128))

                # qk matmuls; bias folded into the DVE evict (per-partition add)
                for m in range(4):
                    pqk = ps_a.tile([128, TCH], F32, tag="a")
                    for s in range(8):
                        nc.tensor.matmul(pqk[:], wqk_sb[:, m, s, :],
                                         xt_ch[:, s, :], start=(s == 0),
                                         stop=(s == 7))
                    tQr = work.tile([128, TCH], BF16, tag="tQr")
                    nc.vector.tensor_scalar_add(tQr[:], pqk[:],
                                                bqk_sb[:, m:m + 1])
                    tQc = work.tile([128, TCH], BF16, tag="tQc")
                    nc.vector.tensor_mul(tQc[:], tQr[:], cos_sb[:, tcols])
                    tQs = work.tile([128, TCH], BF16, tag="tQs")
                    for (a0, a1, b0, b1) in swap:
                        nc.vector.tensor_mul(tQs[a0:a1, :], tQr[b0:b1, :],
                                             sin_sb[b0:b1, tcols])
                    dest = (qT if m % 2 == 0 else kT)[m // 2]
                    nc.vector.tensor_add(dest[:, tcols], tQc[:], tQs[:])

                # v matmuls (natural orientation; bias folded into host-side
                # output correction since sum(att) == 1) + evict into v_sb
                for ts in range(4):
                    pv = ps_a.tile([128, 256], F32, tag="a")
                    for s in range(8):
                        nc.tensor.matmul(pv[:], xt_ch[:, s, ts * 128:(ts + 1) * 128],
                                         wv_sb[:, s, :], start=(s == 0),
                                         stop=(s == 7))
                    sl = tcH * 4 + ts
                    nc.scalar.activation(
                        v_sb[:, sl, :, 0:64],
                        pv[:].rearrange("p (h e) -> p h e", e=64), AF.Copy)

                # ======== attention for q-chunk tcH ========
                yT_ch = yp.tile([128, 2, TCH], BF16, tag="yT")
                nslabs = 4 * tcH + 4

                for p in range(2):
                    pos = [ps_o.tile([128, TCH], F32, tag="O", name=f"po{hh}")
                           for hh in range(2)]

                    def emit_S(j):
                        rr = j - 4 * tcH
                        r = max(rr, 0) * 128
                        qs = slice(tcH * TCH + r, (tcH + 1) * TCH)
                        psS = ps_s.tile([128, 2, TCH], F32, tag="S")
                        for hh in range(2):
                            base = 64 * hh
                            nc.tensor.matmul(
                                psS[:, hh, r:TCH],
                                kT[p][base:base + 64, j * 128:(j + 1) * 128],
                                qT[p][base:base + 64, qs],
                                start=True, stop=True)
                        expS = ep.tile([128, 2, TCH], BF16, tag="expS")
                        nc.scalar.activation(expS[:, :, r:TCH], psS[:, :, r:TCH],
                                             AF.Exp, scale=0.125)
                        if rr >= 0:
                            for hh in range(2):
                                nc.vector.tensor_mul(expS[:, hh, r:r + 128],
                                                     expS[:, hh, r:r + 128],
                                                     msk_sb[:, :])
                        return expS, r

                    def emit_PV(j, expS, r, hh):
                        h = 2 * p + hh
                        nc.tensor.matmul(pos[hh][0:65, r:TCH],
                                         v_sb[:, j, h, :],
                                         expS[:, hh, r:TCH],
                                         start=(j == 0),
                                         stop=(j == nslabs - 1))

                    def emit_norm(hh):
                        base, po = 64 * hh, pos[hh]
                        l_r = work.tile([1, TCH], F32R, tag="lr")
                        with nc.allow_low_precision(reason="f32r rounding of 1/l"):
                            nc.vector.reciprocal(l_r[:], po[64:65, :])
                        lbc = work.tile([64, TCH], F32R, tag="lbc")
                        nc.gpsimd.partition_broadcast(lbc[:], l_r[:])
                        nc.vector.tensor_mul(yT_ch[base:base + 64, p, :],
                                             po[0:64, :], lbc[:])

                    # software pipeline depth 2: S(j+2) issued before PV(j) so
                    # exp latency and the po-tile handover are hidden by PE work
                    win = [emit_S(0)]
                    if nslabs > 1:
                        win.append(emit_S(1))
                    for j in range(nslabs):
                        if j + 2 < nslabs:
                            win.append(emit_S(j + 2))
                        expS, r = win[0]
                        if j == nslabs - 1:
                            # finish each head's PV then immediately start its
                            # normalize chain to overlap the other head's PV
                            emit_PV(j, expS, r, 0)
                            emit_norm(0)
                            emit_PV(j, expS, r, 1)
                            emit_norm(1)
                        else:
                            emit_PV(j, expS, r, 0)
                            emit_PV(j, expS, r, 1)
                        win.pop(0)

                # ======== proj for q-chunk tcH ========
                for mp in range(4):
                    pp = ps_s.tile([128, 2, TCH], F32, tag="S")
                    for half in range(2):
                        mo = 2 * mp + half
                        for s in range(2):
                            nc.tensor.matmul(pp[:, half, :],
                                             wp_sb[:, s, mo * 128:(mo + 1) * 128],
                                             yT_ch[:, s, :], start=(s == 0),
                                             stop=(s == 1))
                    o_st = ost.tile([128, 2, TCH], BF16, tag="ost")
                    if mp % 2 == 0:
                        nc.vector.tensor_copy(o_st[:], pp[:])
                    else:
                        nc.scalar.activation(o_st[:], pp[:], AF.Copy)
                    for half in range(2):
                        mo = 2 * mp + half
                        nc.sync.dma_start(out_t[mo * 128:(mo + 1) * 128, tcols],
                                          o_st[:, half, :])

    nc.compile()
    return nc


def _rope_tables():
    pos = np.arange(T, dtype=np.float32)[:, None]                  # [T, 1]
    i = np.arange(1, HS // 2 + 1, dtype=np.float32)[None]          # [1, 32]
    theta = 1.0 / 10000.0 ** (2.0 * (i - 1.0) / HS)
    ang = pos * theta
    cos, sin = np.cos(ang).T, np.sin(ang).T                        # [32, T]
    cos_rep = np.tile(cos, (4, 1)).astype(np.float32)              # [128, T]
    sin_sgn = np.concatenate([sin, -sin, sin, -sin], 0).astype(np.float32)
    return cos_rep, sin_sgn


def _mask128():
    p = np.arange(128)[:, None]
    f = np.arange(128)[None, :]
    return (p <= f).astype(np.float32)


def kernel(x, W_qkv, b_qkv, W_proj, b_proj):
    global last_results
    import ml_dtypes
    from concourse.bass_utils import run_bass_kernel_spmd

    bf16 = ml_dtypes.bfloat16

    if "nc" not in _cache:
        _cache["nc"] = _build()
    nc = _cache["nc"]

    x = np.asarray(x, np.float32)
    W_qkv = np.asarray(W_qkv, np.float32)
    b_qkv = np.asarray(b_qkv, np.float32)
    W_proj = np.asarray(W_proj, np.float32)
    b_proj = np.asarray(b_proj, np.float32)

    perm = np.concatenate([np.arange(0, HS, 2), np.arange(1, HS, 2)])  # even|odd
    cos_rep, sin_sgn = _rope_tables()
    cmask = _mask128()

    in_maps = []
    for core in range(NCORES):
        b, g = core // 4, core % 4
        heads = [4 * g + j for j in range(HPC)]
        wq = [W_qkv[:, h * 3 * HS:h * 3 * HS + HS][:, perm] for h in heads]
        wk = [W_qkv[:, h * 3 * HS + HS:h * 3 * HS + 2 * HS][:, perm] for h in heads]
        wv_ = [W_qkv[:, h * 3 * HS + 2 * HS:h * 3 * HS + 3 * HS] for h in heads]
        bq = [b_qkv[h * 3 * HS:h * 3 * HS + HS][perm] for h in heads]
        bk = [b_qkv[h * 3 * HS + HS:h * 3 * HS + 2 * HS][perm] for h in heads]
        bv_ = [b_qkv[h * 3 * HS + 2 * HS:h * 3 * HS + 3 * HS] for h in heads]
        # col-chunks: [q01 | k01 | q23 | k23]
        wqk = np.concatenate([wq[0], wq[1], wk[0], wk[1],
                              wq[2], wq[3], wk[2], wk[3]], axis=1)
        bqk = np.concatenate([bq[0], bq[1], bk[0], bk[1],
                              bq[2], bq[3], bk[2], bk[3]])
        in_maps.append({
            "xt_in": np.ascontiguousarray(x[b].T).astype(bf16),
            "wqk": np.ascontiguousarray(wqk).astype(bf16),
            "bqk_c": np.ascontiguousarray(bqk.reshape(4, 128).T),
            "wv": np.ascontiguousarray(np.concatenate(wv_, axis=1)).astype(bf16),
            "wp": np.ascontiguousarray(W_proj[g * 256:(g + 1) * 256, :]).astype(bf16),
            "cos_in": cos_rep.astype(bf16),
            "sin_in": sin_sgn.astype(bf16),
            "cmask": cmask.astype(bf16),
        })

    res = run_bass_kernel_spmd(nc, in_maps, core_ids=list(range(NCORES)))
    last_results = res

    out = np.zeros((B, T, C), dtype=np.float32)
    for core in range(NCORES):
        b = core // 4
        out[b] += res.results[core]["out_t"].astype(np.float32).T
    # v-bias shifts y by exactly bv per head (sum(att) == 1), so its effect
    # on the output is the constant bv_full @ W_proj
    bv_full = np.concatenate(
        [b_qkv[h * 3 * HS + 2 * HS:h * 3 * HS + 3 * HS] for h in range(H)])
    out += (b_proj + bv_full @ W_proj)[None, None, :]
    return out
